# revision 1
# baseline (speedup 1.0000x reference)
"""Trainium2 Bass kernel for ConvReverseDataNet (USRNet-style FFT data step).

Math (per (b,c) plane, sf=2, validated vs reference in fp32):
  g   = fft2_128(x)                                (128x128 complex)
  FB  = G @ k @ G^T, G = F256[:, roll_idx]         (256x256 complex)
  W   = sum_{4 blocks} |FB|^2 ;  Y0 = sum_{4 blocks} FB*DD   (128x128)
  wt  = (4 - Y0) / (W + 4*be)                      (128x128 complex)
  FX  = tile(g) * (conj(FB)*tile(wt) + DD)         (256x256 complex)
  out = real(ifft2_256(FX))                         = Fc@FX@Fc / 65536
where DD = outer(d, d), d[t] = 1 + exp(-2*pi*i*t/256), be = sigmoid(alpha-9)+1e-3.
All complex arrays X are stored as (Xr, Xs) with X = Xr - i*Xs.
256x256 planes live in SBUF as [128, 512]: [p, hb*256+f] = plane[hb*128+p, f].

Sharding: 256 (b,c) planes over 8 cores; core ci gets channels ci*8..ci*8+7 x all 4 batches.
"""

import functools
import sys

import numpy as np

if "/opt/trn_rl_repo" not in sys.path:
    sys.path.insert(0, "/opt/trn_rl_repo")

from concourse import bacc, bass, mybir, tile  # noqa: E402
from concourse.bass_utils import run_bass_kernel_spmd  # noqa: E402

F32 = mybir.dt.float32
MULT = mybir.AluOpType.mult
ADD = mybir.AluOpType.add

N_CORES = 8
NPL = 32  # planes per core
KS = 25


def _host_consts():
    t1 = np.arange(128)
    th1 = 2 * np.pi * np.outer(t1, t1) / 128
    C1 = np.cos(th1).astype(np.float32)
    S1 = np.sin(th1).astype(np.float32)
    t2 = np.arange(256)
    th2 = 2 * np.pi * np.outer(t2, t2) / 256
    C2 = np.cos(th2)
    S2 = np.sin(th2)
    idx = (np.arange(KS) - (KS // 2)) % 256
    GcT = C2[idx, :].astype(np.float32)  # [25,256]
    GsT = S2[idx, :].astype(np.float32)
    # Cnat[p, kc*256+m] = C2[kc*128+p, m]
    Cnat = C2.reshape(2, 128, 256).transpose(1, 0, 2).reshape(128, 512).astype(np.float32)
    Snat = S2.reshape(2, 128, 256).transpose(1, 0, 2).reshape(128, 512).astype(np.float32)
    dr = 1 + np.cos(2 * np.pi * t2 / 256)
    ds = np.sin(2 * np.pi * t2 / 256)

    def to_plane(a):
        return a.reshape(2, 128, 256).transpose(1, 0, 2).reshape(128, 512)

    DDr = to_plane(np.outer(dr, dr) - np.outer(ds, ds)).astype(np.float32)
    DDs = to_plane(np.outer(dr, ds) + np.outer(ds, dr)).astype(np.float32)
    return {
        "C1": C1, "S1": S1, "S1n": -S1,
        "GcT": GcT, "GsT": GsT, "GsTn": -GsT,
        "Cnat": Cnat, "Snat": Snat, "Snatn": -Snat,
        "DDr": DDr, "DDs": DDs,
    }


CONST_SHAPES = {
    "C1": [128, 128], "S1": [128, 128], "S1n": [128, 128],
    "GcT": [KS, 256], "GsT": [KS, 256], "GsTn": [KS, 256],
    "Cnat": [128, 512], "Snat": [128, 512], "Snatn": [128, 512],
    "DDr": [128, 512], "DDs": [128, 512],
}


def build_nc(n_planes=NPL):
    nc = bacc.Bacc("TRN2", target_bir_lowering=False, debug=False, num_devices=N_CORES)

    xs_t = nc.dram_tensor("xs", [n_planes, 128, 128], F32, kind="ExternalInput")
    kt_t = nc.dram_tensor("kt", [n_planes, KS, KS], F32, kind="ExternalInput")
    be4_t = nc.dram_tensor("be4", [128, n_planes], F32, kind="ExternalInput")
    const_t = {n: nc.dram_tensor(n, s, F32, kind="ExternalInput") for n, s in CONST_SHAPES.items()}
    out_t = nc.dram_tensor("out", [n_planes, 256, 256], F32, kind="ExternalOutput")

    with tile.TileContext(nc) as tc:
        with (
            tc.tile_pool(name="cpool", bufs=1) as cpool,
            tc.tile_pool(name="small", bufs=3) as small,
            tc.tile_pool(name="big", bufs=2) as big,
            tc.tile_pool(name="psum", bufs=1, space="PSUM") as pp,
        ):
            cs = {}
            for n, s in CONST_SHAPES.items():
                cs[n] = cpool.tile(s, F32, tag=n, name=f"c_{n}")
                nc.sync.dma_start(cs[n][:], const_t[n][:])
            be4sb = cpool.tile([128, n_planes], F32, tag="be4sb")
            nc.sync.dma_start(be4sb[:], be4_t[:])

            def b4(ap):  # view flat [128,512] as [128,4,128]
                return ap.rearrange("p (b f) -> p b f", b=4)

            def rep4(ap128):  # [128,128] -> broadcast [128,4,128]
                return ap128.unsqueeze(1).broadcast_to([128, 4, 128])

            for i in range(n_planes):
                # ---- loads ----
                x_sb = small.tile([128, 128], F32, tag="x_sb")
                nc.sync.dma_start(x_sb[:], xs_t[i])
                kt_sb = small.tile([KS, KS], F32, tag="kt_sb")
                nc.sync.dma_start(kt_sb[:], kt_t[i])

                # ---- fft128: g = F1 @ x @ F1 ----
                z_sb = small.tile([128, 256], F32, tag="z_sb")  # Zrt | Zst
                pzr = pp.tile([128, 128], F32, tag="p128", bufs=2)
                nc.tensor.matmul(pzr[:], x_sb[:], cs["C1"][:], start=True, stop=True)
                nc.scalar.copy(z_sb[:, 0:128], pzr[:])
                pzs = pp.tile([128, 128], F32, tag="p128", bufs=2)
                nc.tensor.matmul(pzs[:], x_sb[:], cs["S1"][:], start=True, stop=True)
                nc.scalar.copy(z_sb[:, 128:256], pzs[:])

                g_sb = small.tile([128, 256], F32, tag="g_sb")  # gr | gs
                pgr = pp.tile([128, 128], F32, tag="p128", bufs=2)
                nc.tensor.matmul(pgr[:], z_sb[:, 0:128], cs["C1"][:], start=True, stop=False)
                nc.tensor.matmul(pgr[:], z_sb[:, 128:256], cs["S1n"][:], start=False, stop=True)
                nc.scalar.copy(g_sb[:, 0:128], pgr[:])
                pgs = pp.tile([128, 128], F32, tag="p128", bufs=2)
                nc.tensor.matmul(pgs[:], z_sb[:, 0:128], cs["S1"][:], start=True, stop=False)
                nc.tensor.matmul(pgs[:], z_sb[:, 128:256], cs["C1"][:], start=False, stop=True)
                nc.scalar.copy(g_sb[:, 128:256], pgs[:])

                # ---- FB = G @ k @ G^T ----
                a_sb = small.tile([KS, 512], F32, tag="a_sb")  # Ar | As
                par = pp.tile([KS, 256], F32, tag="pa")
                nc.tensor.matmul(par[:], kt_sb[:], cs["GcT"][:], start=True, stop=True)
                nc.scalar.copy(a_sb[:, 0:256], par[:])
                pas = pp.tile([KS, 256], F32, tag="pa")
                nc.tensor.matmul(pas[:], kt_sb[:], cs["GsT"][:], start=True, stop=True)
                nc.scalar.copy(a_sb[:, 256:512], pas[:])

                FBr = big.tile([128, 512], F32, tag="FBr")
                FBs = big.tile([128, 512], F32, tag="FBs")
                for hh in range(2):
                    hsl = slice(hh * 128, (hh + 1) * 128)
                    pfbr = pp.tile([128, 256], F32, tag="pfb", bufs=2)
                    nc.tensor.matmul(pfbr[:], cs["GcT"][:, hsl], a_sb[:, 0:256], start=True, stop=False)
                    nc.tensor.matmul(pfbr[:], cs["GsTn"][:, hsl], a_sb[:, 256:512], start=False, stop=True)
                    nc.scalar.copy(FBr[:, hh * 256:(hh + 1) * 256], pfbr[:])
                    pfbs = pp.tile([128, 256], F32, tag="pfb", bufs=2)
                    nc.tensor.matmul(pfbs[:], cs["GcT"][:, hsl], a_sb[:, 256:512], start=True, stop=False)
                    nc.tensor.matmul(pfbs[:], cs["GsT"][:, hsl], a_sb[:, 0:256], start=False, stop=True)
                    nc.scalar.copy(FBs[:, hh * 256:(hh + 1) * 256], pfbs[:])

                # ---- elementwise: W, Y0, wt ----
                sq1 = big.tile([128, 512], F32, tag="sq1")
                sq2 = big.tile([128, 512], F32, tag="sq2")
                nc.scalar.square(sq1[:], FBr[:])
                nc.scalar.square(sq2[:], FBs[:])
                F2B = big.tile([128, 512], F32, tag="F2B")
                nc.vector.tensor_add(F2B[:], sq1[:], sq2[:])

                m1 = big.tile([128, 512], F32, tag="m1")
                m2 = big.tile([128, 512], F32, tag="m2")
                nc.vector.tensor_mul(m1[:], FBr[:], cs["DDr"][:])
                nc.vector.scalar_tensor_tensor(m2[:], FBs[:], -1.0, cs["DDs"][:], MULT, MULT)
                Pr0 = big.tile([128, 512], F32, tag="Pr0")
                nc.vector.tensor_add(Pr0[:], m1[:], m2[:])
                m3 = big.tile([128, 512], F32, tag="m3")
                m4 = big.tile([128, 512], F32, tag="m4")
                nc.vector.tensor_mul(m3[:], FBr[:], cs["DDs"][:])
                nc.vector.tensor_mul(m4[:], FBs[:], cs["DDr"][:])
                Ps0 = big.tile([128, 512], F32, tag="Ps0")
                nc.vector.tensor_add(Ps0[:], m3[:], m4[:])

                sums = {}
                for nm, src in (("W", F2B), ("Yr0", Pr0), ("Ys0", Ps0)):
                    sA = small.tile([128, 256], F32, tag=f"sA_{nm}")
                    v = src[:].rearrange("p (a b f) -> p a b f", a=2, b=2)
                    nc.vector.tensor_add(sA[:].rearrange("p (a f) -> p a f", a=2), v[:, :, 0, :], v[:, :, 1, :])
                    dst = small.tile([128, 128], F32, tag=nm)
                    nc.vector.tensor_add(dst[:], sA[:, 0:128], sA[:, 128:256])
                    sums[nm] = dst

                den = small.tile([128, 128], F32, tag="den")
                nc.vector.tensor_scalar_add(den[:], sums["W"][:], be4sb[:, i:i + 1])
                dinv = small.tile([128, 128], F32, tag="dinv")
                nc.vector.reciprocal_approx_fast(dinv[:], den[:])
                wt4 = small.tile([128, 128], F32, tag="wt4")
                nc.vector.tensor_scalar(wt4[:], sums["Yr0"][:], -1.0, 4.0, MULT, ADD)
                wr = small.tile([128, 128], F32, tag="wr")
                nc.vector.tensor_mul(wr[:], wt4[:], dinv[:])
                ws = small.tile([128, 128], F32, tag="ws")
                nc.vector.scalar_tensor_tensor(ws[:], sums["Ys0"][:], -1.0, dinv[:], MULT, MULT)

                # ---- H = conj(FB) * tile(wt) + DD ----
                twr = rep4(wr[:])
                tws = rep4(ws[:])
                p1 = big.tile([128, 512], F32, tag="p1")
                p2 = big.tile([128, 512], F32, tag="p2")
                nc.vector.tensor_mul(b4(p1[:]), b4(FBr[:]), twr)
                nc.vector.tensor_mul(b4(p2[:]), b4(FBs[:]), tws)
                s12 = big.tile([128, 512], F32, tag="s12")
                nc.vector.tensor_add(s12[:], p1[:], p2[:])
                Hr = big.tile([128, 512], F32, tag="Hr")
                nc.vector.tensor_add(Hr[:], s12[:], cs["DDr"][:])
                p3 = big.tile([128, 512], F32, tag="p3")
                p4 = big.tile([128, 512], F32, tag="p4")
                nc.vector.tensor_mul(b4(p3[:]), b4(FBr[:]), tws)
                nc.vector.scalar_tensor_tensor(b4(p4[:]), b4(FBs[:]), -1.0, twr, MULT, MULT)
                s34 = big.tile([128, 512], F32, tag="s34")
                nc.vector.tensor_add(s34[:], p3[:], p4[:])
                Hs = big.tile([128, 512], F32, tag="Hs")
                nc.vector.tensor_add(Hs[:], s34[:], cs["DDs"][:])

                # ---- FX = tile(g) * H ----
                tgr = rep4(g_sb[:, 0:128])
                tgs = rep4(g_sb[:, 128:256])
                q1 = big.tile([128, 512], F32, tag="q1")
                q2 = big.tile([128, 512], F32, tag="q2")
                nc.vector.tensor_mul(b4(q1[:]), b4(Hr[:]), tgr)
                nc.vector.scalar_tensor_tensor(b4(q2[:]), b4(Hs[:]), -1.0, tgs, MULT, MULT)
                FXr = big.tile([128, 512], F32, tag="FXr")
                nc.vector.tensor_add(FXr[:], q1[:], q2[:])
                q3 = big.tile([128, 512], F32, tag="q3")
                q4 = big.tile([128, 512], F32, tag="q4")
                nc.vector.tensor_mul(b4(q3[:]), b4(Hs[:]), tgr)
                nc.vector.tensor_mul(b4(q4[:]), b4(Hr[:]), tgs)
                FXs = big.tile([128, 512], F32, tag="FXs")
                nc.vector.tensor_add(FXs[:], q3[:], q4[:])

                # ---- ifft stage 1: VT ----
                VTr = big.tile([128, 512], F32, tag="VTr")
                VTs = big.tile([128, 512], F32, tag="VTs")
                for fb in range(2):
                    pvtr = pp.tile([128, 256], F32, tag="pvt", bufs=2)
                    pvts = pp.tile([128, 256], F32, tag="pvt", bufs=2)
                    for kc in range(2):
                        lsl = slice(kc * 256 + fb * 128, kc * 256 + (fb + 1) * 128)
                        csl = slice(kc * 256, (kc + 1) * 256)
                        st = kc == 0
                        nc.tensor.matmul(pvtr[:], FXr[:, lsl], cs["Cnat"][:, csl], start=st, stop=False)
                        nc.tensor.matmul(pvtr[:], FXs[:, lsl], cs["Snat"][:, csl], start=False, stop=(kc == 1))
                    for kc in range(2):
                        lsl = slice(kc * 256 + fb * 128, kc * 256 + (fb + 1) * 128)
                        csl = slice(kc * 256, (kc + 1) * 256)
                        st = kc == 0
                        nc.tensor.matmul(pvts[:], FXs[:, lsl], cs["Cnat"][:, csl], start=st, stop=False)
                        nc.tensor.matmul(pvts[:], FXr[:, lsl], cs["Snatn"][:, csl], start=False, stop=(kc == 1))
                    nc.scalar.copy(VTr[:, fb * 256:(fb + 1) * 256], pvtr[:])
                    nc.scalar.copy(VTs[:, fb * 256:(fb + 1) * 256], pvts[:])

                # ---- ifft stage 2 (real part) + scale + store ----
                out_sb = big.tile([128, 512], F32, tag="out_sb")
                po = pp.tile([128, 512], F32, tag="po")
                for mb in range(2):
                    osl = slice(mb * 256, (mb + 1) * 256)
                    for fb in range(2):
                        lsl = slice(fb * 256 + mb * 128, fb * 256 + (mb + 1) * 128)
                        csl = slice(fb * 256, (fb + 1) * 256)
                        nc.tensor.matmul(po[:, osl], VTr[:, lsl], cs["Cnat"][:, csl], start=(fb == 0), stop=False)
                        nc.tensor.matmul(po[:, osl], VTs[:, lsl], cs["Snat"][:, csl], start=False, stop=(fb == 1))
                    nc.scalar.mul(out_sb[:, osl], po[:, osl], 1.0 / 65536.0)
                nc.sync.dma_start(
                    out_t[i].rearrange("(hb p) f -> p hb f", p=128),
                    out_sb[:].rearrange("p (hb f) -> p hb f", hb=2),
                )

    nc.compile()
    return nc


@functools.lru_cache(maxsize=2)
def _built(n_planes=NPL):
    return build_nc(n_planes)


def make_in_maps(x, k, alpha, n_planes=NPL, n_cores=N_CORES):
    consts = _host_consts()
    alpha_c = alpha.reshape(-1).astype(np.float64)  # [64]
    be = (1.0 / (1.0 + np.exp(-(alpha_c - 9.0))) + 1e-3).astype(np.float32)
    cpc = n_planes // 4  # channels per core
    in_maps = []
    for ci in range(n_cores):
        chs = slice(ci * cpc, (ci + 1) * cpc)
        xs = np.ascontiguousarray(x[:, chs].transpose(1, 0, 2, 3).reshape(n_planes, 128, 128))
        kt = np.ascontiguousarray(k[:, chs].transpose(1, 0, 3, 2).reshape(n_planes, KS, KS))
        be_pl = np.repeat(be[chs], 4)  # plane order: (c_loc, b)
        be4 = np.broadcast_to(4.0 * be_pl, (128, n_planes)).astype(np.float32).copy()
        m = {"xs": xs, "kt": kt, "be4": be4}
        m.update(consts)
        in_maps.append(m)
    return in_maps


def kernel(x, k, alpha, sf=2, **_ignored):
    x = np.asarray(x, dtype=np.float32)
    k = np.asarray(k, dtype=np.float32)
    alpha = np.asarray(alpha, dtype=np.float32)
    assert int(sf) == 2 and x.shape == (4, 64, 128, 128) and k.shape == (4, 64, KS, KS)

    nc = _built(NPL)
    in_maps = make_in_maps(x, k, alpha)
    res = run_bass_kernel_spmd(nc, in_maps, core_ids=list(range(N_CORES)))
    out = np.empty((4, 64, 256, 256), np.float32)
    cpc = NPL // 4
    for ci in range(N_CORES):
        o = res.results[ci]["out"].reshape(cpc, 4, 256, 256).transpose(1, 0, 2, 3)
        out[:, ci * cpc:(ci + 1) * cpc] = o
    return out


if __name__ == "__main__":
    rng = np.random.default_rng(0)
    x = rng.standard_normal((4, 64, 128, 128), dtype=np.float32)
    k = rng.random((4, 64, KS, KS), dtype=np.float32)
    alpha = np.zeros((1, 64, 1, 1), np.float32)
    out = kernel(x, k, alpha, 2)
    print("out", out.shape, out.dtype, float(np.abs(out).max()))



# revision 11
# speedup vs baseline: 1.6496x; 1.6496x over previous
"""Trainium2 Bass kernel for ConvReverseDataNet (USRNet-style FFT data step).

Math (per (b,c) plane, sf=2, validated in numpy mock):
  g   = fft2_128(x)                                 (128x128 complex)
  FB  = G @ k @ G^T, G = F256[:, roll_idx]          (256x256 complex)
  W   = sum_{4 blocks} |FB|^2                       (128x128 real)
  Y   = sum_{4 blocks} FB*DD = Ghat @ k @ Ghat^T    (128x128 complex, on PE)
        where Ghat[u,kap] = sum_a d[u+128a] G[u+128a, kap], d[t]=1+exp(-2pi i t/256)
  wt  = (4 - Y) / (W + 4*be)
  u   = g * wt
  FX  = conj(FB) * tile(u)
  out = real(ifft2_256(FX)) + nn_upsample(x)        [ifft2(tile(g)*DD) == xu identity]
All complex X stored as (Xr, Xs) with X = Xr - i*Xs.
256-row planes in SBUF as [128, 512]: [p, hb*256+f] = plane[hb*128+p, f].

Perf notes: matmuls in float32r (1 cyc/row at free>=256) or bf16 (1 cyc/row);
elementwise mostly bf16 on DVE, balanced against Act (copies) and Pool/gpsimd.

Sharding: 256 (b,c) planes over 8 cores; core ci gets channels ci*8..ci*8+7 x 4 batches.
"""

import functools
import sys

import ml_dtypes
import numpy as np

if "/opt/trn_rl_repo" not in sys.path:
    sys.path.insert(0, "/opt/trn_rl_repo")

from concourse import bacc, bass, mybir, tile  # noqa: E402
from concourse.bass_utils import run_bass_kernel_spmd  # noqa: E402

F32 = mybir.dt.float32
F32R = mybir.dt.float32r
BF16 = mybir.dt.bfloat16
MULT = mybir.AluOpType.mult
ADD = mybir.AluOpType.add

N_CORES = 8
NPL = 32  # planes per core
KS = 25


def _host_consts():
    t1 = np.arange(128)
    th1 = 2 * np.pi * np.outer(t1, t1) / 128
    C1 = np.cos(th1)
    S1 = np.sin(th1)
    t2 = np.arange(256)
    th2 = 2 * np.pi * np.outer(t2, t2) / 256
    C2 = np.cos(th2)
    S2 = np.sin(th2)
    idx = (np.arange(KS) - (KS // 2)) % 256
    GcT = C2[idx, :]  # [25,256]
    GsT = S2[idx, :]

    def to_plane(a):  # [256,256] -> [128,512]
        return a.reshape(2, 128, 256).transpose(1, 0, 2).reshape(128, 512)

    Cnat = to_plane(C2)
    Snat = to_plane(S2)

    consts = {}
    consts["CS1"] = np.concatenate([C1, S1], 1)              # [128,256]
    consts["CS2"] = np.concatenate([-S1, C1], 1)             # [128,256]
    consts["GT"] = np.concatenate([GcT, GsT], 1)             # [25,512]
    zpad = np.zeros((7, 128))
    for hh in range(2):
        hsl = slice(hh * 128, (hh + 1) * 128)
        consts[f"GGr{hh}"] = np.concatenate([GcT[:, hsl], zpad, -GsT[:, hsl]], 0)  # [57,128]
        consts[f"GGs{hh}"] = np.concatenate([GsT[:, hsl], zpad, GcT[:, hsl]], 0)
    # Ghat for Y
    d = 1 + np.exp(-2j * np.pi * t2 / 256)
    G = np.exp(-2j * np.pi * np.outer(t2, idx) / 256)        # [256,25]
    Ghat = (d[:128, None] * G[:128, :]) + (d[128:, None] * G[128:, :])  # [128,25]
    Ghc, Ghs = np.real(Ghat), -np.imag(Ghat)                 # Ghat = Ghc - i*Ghs
    consts["GhT2"] = np.concatenate([Ghc.T, Ghs.T], 1)       # [25,256]
    consts["Kr"] = np.concatenate([Ghc.T, zpad, -Ghs.T], 0)  # [57,128] -> bf16
    consts["Ks"] = np.concatenate([Ghs.T, zpad, Ghc.T], 0)   # [57,128] -> bf16
    # VT moving blocks
    vm1, vm2 = [], []
    for kc in range(2):
        c = Cnat[:, kc * 256:(kc + 1) * 256]
        s = Snat[:, kc * 256:(kc + 1) * 256]
        vm1.append(np.concatenate([c, -s], 1))
        vm2.append(np.concatenate([s, c], 1))
    consts["VM1"] = np.concatenate(vm1, 1)                   # [128,1024] -> bf16
    consts["VM2"] = np.concatenate(vm2, 1)
    consts["CnS"] = Cnat / 65536.0                           # [128,512] -> bf16
    consts["SnS"] = Snat / 65536.0
    # xu row-replication selectors [128,256] = [R0 | R1]
    R = np.zeros((2, 128, 128))
    for mb in range(2):
        for p in range(128):
            R[mb, mb * 64 + p // 2, p] = 1.0
    consts["R2"] = np.concatenate([R[0], R[1]], 1)           # [128,256]
    consts["ZP256"] = np.zeros((7, 256))
    consts["ZP128"] = np.zeros((7, 128))
    return consts


CONST_SPECS = {
    "CS1": ([128, 256], F32R), "CS2": ([128, 256], F32R),
    "GT": ([25, 512], F32R),
    "GGr0": ([57, 128], F32R), "GGs0": ([57, 128], F32R),
    "GGr1": ([57, 128], F32R), "GGs1": ([57, 128], F32R),
    "GhT2": ([25, 256], F32R),
    "Kr": ([57, 128], BF16), "Ks": ([57, 128], BF16),
    "VM1": ([128, 1024], F32R), "VM2": ([128, 1024], F32R),
    "CnS": ([128, 512], F32R), "SnS": ([128, 512], F32R),
    "R2": ([128, 256], F32R),
    "ZP256": ([7, 256], F32R),
    "ZP128": ([7, 128], BF16),
}


def build_nc(n_planes=NPL, debug=False):
    nc = bacc.Bacc("TRN2", target_bir_lowering=False, debug=False, num_devices=N_CORES)

    xs_t = nc.dram_tensor("xs", [n_planes, 128, 128], F32R, kind="ExternalInput")
    kt_t = nc.dram_tensor("kt", [n_planes, KS, KS], F32R, kind="ExternalInput")
    be4_t = nc.dram_tensor("be4", [128, n_planes], F32, kind="ExternalInput")
    const_t = {n: nc.dram_tensor(n, s, dt, kind="ExternalInput") for n, (s, dt) in CONST_SPECS.items()}
    out_t = nc.dram_tensor("out", [n_planes, 256, 256], F32, kind="ExternalOutput")
    dbg_t = {}
    if debug:
        for nm, shp, dt in [("g16", [128, 256], BF16), ("a_st", [57, 256], F32),
                            ("a2st", [57, 128], BF16), ("FB16", [128, 1024], BF16),
                            ("Wp", [128, 128], F32), ("wr16", [128, 128], BF16),
                            ("ws16", [128, 128], BF16), ("u16", [128, 384], BF16),
                            ("FX16", [128, 1024], F32), ("VT16", [128, 1024], F32),
                            ("xc", [128, 256], F32), ("z_sb", [128, 256], F32)]:
            dbg_t[nm] = nc.dram_tensor(f"dbg_{nm}", shp, dt, kind="ExternalOutput")

    r = lambda ap: ap.bitcast(F32R)

    with tile.TileContext(nc) as tc:
        with (
            tc.tile_pool(name="cpool", bufs=1) as cpool,
            tc.tile_pool(name="small", bufs=3) as small,
            tc.tile_pool(name="big", bufs=2) as big,
            tc.tile_pool(name="psA", bufs=1, space="PSUM") as psA,
            tc.tile_pool(name="psB", bufs=2, space="PSUM") as psB,
        ):
            cs = {}
            for n, (s, dt) in CONST_SPECS.items():
                cs[n] = cpool.tile(s, dt, tag=n, name=f"c_{n}")
                nc.sync.dma_start(cs[n][:], const_t[n][:])
            be4sb = cpool.tile([128, n_planes], F32, tag="be4sb")
            nc.sync.dma_start(be4sb[:], be4_t[:])

            for i in range(n_planes):
                # ---- loads ----
                x_sb = small.tile([128, 128], F32R, tag="x_sb")
                nc.sync.dma_start(x_sb[:], xs_t[i])
                kt_sb = small.tile([KS, KS], F32R, tag="kt_sb")
                nc.sync.dma_start(kt_sb[:], kt_t[i])

                # ---- fft128: pzg bank = [Z | PG] is too big; use [128,512] = [pz|pg] ----
                pzg = psB.tile([128, 512], F32, tag="pzg_po")  # shared rotation with po
                pz = pzg[:, 0:256]
                pg = pzg[:, 256:512]
                nc.tensor.matmul(pz, r(x_sb[:]), r(cs["CS1"][:]), start=True, stop=True)
                z_sb = small.tile([128, 256], F32R, tag="z_sb")
                nc.scalar.copy(z_sb[:], pz)
                nc.tensor.matmul(pg, r(z_sb[:, 0:128]), r(cs["CS1"][:]), start=True, stop=False)
                nc.tensor.matmul(pg, r(z_sb[:, 128:256]), r(cs["CS2"][:]), start=False, stop=True)
                g16 = small.tile([128, 256], BF16, tag="g16")  # [gr|gs]
                nc.vector.tensor_copy(g16[:], pg)

                # ---- FB = G k G^T ----
                pa = psA.tile([25, 512], F32, tag="pa")
                nc.tensor.matmul(pa[:], r(kt_sb[:]), r(cs["GT"][:]), start=True, stop=True)
                a_st = small.tile([57, 256], F32R, tag="a_st")  # rows [Ar; 0pad; As]
                nc.sync.dma_start(a_st[25:32, :], cs["ZP256"][:])
                nc.scalar.copy(a_st[0:25, :], pa[:, 0:256])
                nc.scalar.copy(a_st[32:57, :], pa[:, 256:512])

                # ---- Y = Ghat k Ghat^T ----
                pY2 = psA.tile([128, 512], F32, tag="pY2")  # [0:25,0:256]=pa2, [:,256:512]=pY
                pa2 = pY2[0:25, 0:256]
                pY = pY2[:, 256:512]
                nc.tensor.matmul(pa2, r(kt_sb[:]), r(cs["GhT2"][:]), start=True, stop=True)
                a2st = small.tile([57, 128], BF16, tag="a2st")  # rows [A2r; 0pad; A2s]
                nc.sync.dma_start(a2st[25:32, :], cs["ZP128"][:])
                nc.vector.tensor_copy(a2st[0:25, :], pY2[0:25, 0:128])
                nc.vector.tensor_copy(a2st[32:57, :], pY2[0:25, 128:256])
                nc.tensor.matmul(pY[:, 0:128], cs["Kr"][:], a2st[:], start=True, stop=True)
                nc.tensor.matmul(pY[:, 128:256], cs["Ks"][:], a2st[:], start=True, stop=True)

                # FB16 [128,1024] = [FBr_h0 | FBs_h0 | FBr_h1 | FBs_h1]
                FB16 = big.tile([128, 1024], BF16, tag="FB16")
                for hh in range(2):
                    pfb = psB.tile([128, 512], F32, tag="pfb")
                    nc.tensor.matmul(pfb[:, 0:256], r(cs[f"GGr{hh}"][:]), r(a_st[:]), start=True, stop=True)
                    nc.tensor.matmul(pfb[:, 256:512], r(cs[f"GGs{hh}"][:]), r(a_st[:]), start=True, stop=True)
                    nc.scalar.copy(FB16[:, hh * 512:(hh + 1) * 512], pfb[:])

                # ---- W = sum4 |FB|^2 ----
                sq = big.tile([128, 1024], BF16, tag="sq")
                nc.vector.tensor_mul(sq[:], FB16[:], FB16[:])
                tt = big.tile([128, 512], BF16, tag="tt")
                sqv = sq[:].rearrange("p (h c f) -> p h c f", h=2, c=2)
                nc.vector.tensor_add(tt[:].rearrange("p (h f) -> p h f", h=2), sqv[:, :, 0, :], sqv[:, :, 1, :])
                w2 = small.tile([128, 256], BF16, tag="w2")
                ttv = tt[:].rearrange("p (h v f) -> p h v f", h=2, v=2)
                nc.vector.tensor_add(w2[:].rearrange("p (h f) -> p h f", h=2), ttv[:, :, 0, :], ttv[:, :, 1, :])
                Wp = small.tile([128, 128], F32, tag="Wp")
                nc.vector.tensor_add(Wp[:], w2[:, 0:128], w2[:, 128:256])

                # ---- wt = (4 - Y)/(W + 4be) ----
                den = small.tile([128, 128], F32, tag="den")
                nc.gpsimd.tensor_scalar_add(den[:], Wp[:], be4sb[:, i:i + 1])
                dinv = small.tile([128, 128], F32, tag="dinv")
                nc.vector.reciprocal_approx_fast(dinv[:], den[:])
                wt4 = small.tile([128, 128], F32, tag="wt4")
                nc.vector.tensor_scalar(wt4[:], pY[:, 0:128], -1.0, 4.0, MULT, ADD)
                wr16 = small.tile([128, 128], BF16, tag="wr16")
                nc.vector.tensor_mul(wr16[:], wt4[:], dinv[:])
                ws16 = small.tile([128, 128], BF16, tag="ws16")
                nc.vector.scalar_tensor_tensor(ws16[:], pY[:, 128:256], -1.0, dinv[:], MULT, MULT)

                # ---- u = g*wt:  u16 [128,384] = [ur | us | ur] ----
                u16 = small.tile([128, 384], BF16, tag="u16")
                gr = g16[:, 0:128]
                gs = g16[:, 128:256]
                mu = small.tile([128, 128], BF16, tag="mu")
                nc.vector.tensor_mul(mu[:], gr, wr16[:])
                t1 = small.tile([128, 128], BF16, tag="t1u")
                nc.vector.scalar_tensor_tensor(t1[:], gs, -1.0, ws16[:], MULT, MULT)
                nc.vector.tensor_add(u16[:, 0:128], mu[:], t1[:])   # ur = gr*wr - gs*ws
                nc.gpsimd.tensor_copy(u16[:, 256:384], u16[:, 0:128])
                mu2 = small.tile([128, 128], BF16, tag="mu2")
                nc.vector.tensor_mul(mu2[:], gr, ws16[:])
                t2 = small.tile([128, 128], BF16, tag="t2u")
                nc.vector.tensor_mul(t2[:], gs, wr16[:])
                nc.vector.tensor_add(u16[:, 128:256], mu2[:], t2[:])  # us = gr*ws + gs*wr

                # ---- FX = conj(FB)*tile(u) ----
                # FX16 [128,1024] = [FXr_h0 | FXs_h0 | FXr_h1 | FXs_h1]
                FX16 = big.tile([128, 1024], F32R, tag="FX16")
                P = big.tile([128, 1024], BF16, tag="Pfx")
                Q = big.tile([128, 1024], BF16, tag="Qfx")
                urep = u16[:, 0:256].rearrange("p (c f) -> p c f", c=2)
                urep = urep.unsqueeze(2).broadcast_to([128, 2, 2, 128])
                srep = u16[:, 128:384].rearrange("p (c f) -> p c f", c=2)
                srep = srep.unsqueeze(2).broadcast_to([128, 2, 2, 128])
                for hh in range(2):
                    hb = slice(hh * 512, (hh + 1) * 512)
                    fb_v = FB16[:, hb].rearrange("p (c vb f) -> p c vb f", c=2, vb=2)
                    P_h = P[:, hb]
                    Q_h = Q[:, hb]
                    nc.vector.tensor_tensor(P_h.rearrange("p (c vb f) -> p c vb f", c=2, vb=2), fb_v, urep, MULT)
                    nc.vector.tensor_tensor(Q_h.rearrange("p (c vb f) -> p c vb f", c=2, vb=2), fb_v, srep, MULT)
                    Pv = P_h.rearrange("p (c f) -> p c f", c=2)
                    Qv = Q_h.rearrange("p (c f) -> p c f", c=2)
                    # FXr_h = P[0] + P[1];  FXs_h = Q[0] - Q[1]
                    nc.vector.tensor_add(FX16[:, hh * 512: hh * 512 + 256], Pv[:, 0, :], Pv[:, 1, :])
                    nc.vector.scalar_tensor_tensor(FX16[:, hh * 512 + 256: (hh + 1) * 512], Qv[:, 1, :], -1.0, Qv[:, 0, :], MULT, ADD)

                # ---- ifft stage 1: VT16 [128,1024] = [VTr_f0 | VTs_f0 | VTr_f1 | VTs_f1] ----
                VT16 = big.tile([128, 1024], F32R, tag="VT16")
                for fb in range(2):
                    pvt = psB.tile([128, 512], F32, tag="pvt")
                    for kc in range(2):
                        fxr = FX16[:, kc * 512 + fb * 128: kc * 512 + (fb + 1) * 128]
                        fxs = FX16[:, kc * 512 + 256 + fb * 128: kc * 512 + 256 + (fb + 1) * 128]
                        nc.tensor.matmul(pvt[:], fxr, cs["VM1"][:, kc * 512:(kc + 1) * 512], start=(kc == 0), stop=False)
                        nc.tensor.matmul(pvt[:], fxs, cs["VM2"][:, kc * 512:(kc + 1) * 512], start=False, stop=(kc == 1))
                    nc.scalar.copy(VT16[:, fb * 512:(fb + 1) * 512], pvt[:])

                # ---- ifft stage 2 + xu + store ----
                xc = small.tile([128, 256], F32R, tag="xc")
                nc.vector.tensor_copy(
                    xc[:].rearrange("p (f two) -> p f two", two=2),
                    x_sb[:].unsqueeze(2).broadcast_to([128, 128, 2]),
                )
                po = psB.tile([128, 512], F32, tag="pzg_po")
                for mb in range(2):
                    osl = slice(mb * 256, (mb + 1) * 256)
                    for fb in range(2):
                        vtr = VT16[:, fb * 512 + mb * 128: fb * 512 + (mb + 1) * 128]
                        vts = VT16[:, fb * 512 + 256 + mb * 128: fb * 512 + 256 + (mb + 1) * 128]
                        csl = slice(fb * 256, (fb + 1) * 256)
                        nc.tensor.matmul(po[:, osl], vtr, cs["CnS"][:, csl], start=(fb == 0), stop=False)
                        nc.tensor.matmul(po[:, osl], vts, cs["SnS"][:, csl], start=False, stop=False)
                    nc.tensor.matmul(po[:, osl], r(cs["R2"][:, mb * 128:(mb + 1) * 128]), r(xc[:]), start=False, stop=True)
                out_sb = big.tile([128, 512], F32, tag="out_sb")
                nc.scalar.copy(out_sb[:], po[:])
                nc.sync.dma_start(
                    out_t[i].rearrange("(hb p) f -> p hb f", p=128),
                    out_sb[:].rearrange("p (hb f) -> p hb f", hb=2),
                )
                if debug and i == 0:
                    for nm, t in [("g16", g16), ("a_st", a_st), ("a2st", a2st),
                                  ("FB16", FB16), ("Wp", Wp), ("wr16", wr16),
                                  ("ws16", ws16), ("u16", u16), ("FX16", FX16),
                                  ("VT16", VT16), ("xc", xc), ("z_sb", z_sb)]:
                        nc.sync.dma_start(dbg_t[nm][:], t[:].bitcast(dbg_t[nm].dtype) if t[:].dtype != dbg_t[nm].dtype else t[:])

    nc.compile()
    return nc


@functools.lru_cache(maxsize=2)
def _built(n_planes=NPL):
    return build_nc(n_planes)


def _round_fp32r(a):
    """fp32 -> fp32r (11-bit mantissa, RNE), as float32."""
    u = np.ascontiguousarray(a, dtype=np.float32).view(np.uint32)
    r = (u + 0x7FF + ((u >> 12) & 1)) & np.uint32(0xFFFFF000)
    return r.astype(np.uint32).view(np.float32)


def make_in_maps(x, k, alpha, n_planes=NPL, n_cores=N_CORES):
    consts_f64 = _host_consts()
    consts = {}
    for n, (s, dt) in CONST_SPECS.items():
        a = consts_f64[n]
        if dt == BF16:
            consts[n] = np.ascontiguousarray(a.astype(ml_dtypes.bfloat16))
        elif dt == F32R:
            consts[n] = _round_fp32r(a)
        else:
            consts[n] = np.ascontiguousarray(a, dtype=np.float32)
    alpha_c = alpha.reshape(-1).astype(np.float64)  # [64]
    be = (1.0 / (1.0 + np.exp(-(alpha_c - 9.0))) + 1e-3).astype(np.float32)
    cpc = n_planes // 4  # channels per core
    in_maps = []
    for ci in range(n_cores):
        chs = slice(ci * cpc, (ci + 1) * cpc)
        xs = _round_fp32r(x[:, chs].transpose(1, 0, 2, 3).reshape(n_planes, 128, 128))
        kt = _round_fp32r(k[:, chs].transpose(1, 0, 3, 2).reshape(n_planes, KS, KS))
        be_pl = np.repeat(be[chs], 4)  # plane order: (c_loc, b)
        be4 = np.broadcast_to(4.0 * be_pl, (128, n_planes)).astype(np.float32).copy()
        m = {"xs": xs, "kt": kt, "be4": be4}
        m.update(consts)
        in_maps.append(m)
    return in_maps


def kernel(x, k, alpha, sf=2, **_ignored):
    x = np.asarray(x, dtype=np.float32)
    k = np.asarray(k, dtype=np.float32)
    alpha = np.asarray(alpha, dtype=np.float32)
    assert int(sf) == 2 and x.shape == (4, 64, 128, 128) and k.shape == (4, 64, KS, KS)

    nc = _built(NPL)
    in_maps = make_in_maps(x, k, alpha)
    res = run_bass_kernel_spmd(nc, in_maps, core_ids=list(range(N_CORES)))
    out = np.empty((4, 64, 256, 256), np.float32)
    cpc = NPL // 4
    for ci in range(N_CORES):
        o = res.results[ci]["out"].reshape(cpc, 4, 256, 256).transpose(1, 0, 2, 3)
        out[:, ci * cpc:(ci + 1) * cpc] = o
    return out


if __name__ == "__main__":
    rng = np.random.default_rng(0)
    x = rng.standard_normal((4, 64, 128, 128), dtype=np.float32)
    k = rng.random((4, 64, KS, KS), dtype=np.float32)
    alpha = np.zeros((1, 64, 1, 1), np.float32)
    out = kernel(x, k, alpha, 2)
    print("out", out.shape, out.dtype, float(np.abs(out).max()))


# revision 13
# speedup vs baseline: 1.7089x; 1.0360x over previous
"""Trainium2 Bass kernel for ConvReverseDataNet (USRNet-style FFT data step).

Math (per (b,c) plane, sf=2, validated in numpy mock):
  g   = fft2_128(x)                                 (128x128 complex)
  FB  = G @ k @ G^T, G = F256[:, roll_idx]          (256x256 complex)
  W   = sum_{4 blocks} |FB|^2                       (128x128 real)
  Y   = sum_{4 blocks} FB*DD = Ghat @ k @ Ghat^T    (128x128 complex, on PE)
        where Ghat[u,kap] = sum_a d[u+128a] G[u+128a, kap], d[t]=1+exp(-2pi i t/256)
  wt  = (4 - Y) / (W + 4*be)
  u   = g * wt
  FX  = conj(FB) * tile(u)
  out = real(ifft2_256(FX)) + nn_upsample(x)        [ifft2(tile(g)*DD) == xu identity]
All complex X stored as (Xr, Xs) with X = Xr - i*Xs.
256-row planes in SBUF as [128, 512]: [p, hb*256+f] = plane[hb*128+p, f].

Perf notes: matmuls in float32r (1 cyc/row at free>=256) or bf16 (1 cyc/row);
elementwise mostly bf16 on DVE, balanced against Act (copies) and Pool/gpsimd.

Sharding: 256 (b,c) planes over 8 cores; core ci gets channels ci*8..ci*8+7 x 4 batches.
"""

import functools
import sys

import ml_dtypes
import numpy as np

if "/opt/trn_rl_repo" not in sys.path:
    sys.path.insert(0, "/opt/trn_rl_repo")

from concourse import bacc, bass, mybir, tile  # noqa: E402
from concourse.bass_utils import run_bass_kernel_spmd  # noqa: E402

F32 = mybir.dt.float32
F32R = mybir.dt.float32r
BF16 = mybir.dt.bfloat16
MULT = mybir.AluOpType.mult
ADD = mybir.AluOpType.add

N_CORES = 8
NPL = 32  # planes per core
KS = 25


def _host_consts():
    t1 = np.arange(128)
    th1 = 2 * np.pi * np.outer(t1, t1) / 128
    C1 = np.cos(th1)
    S1 = np.sin(th1)
    t2 = np.arange(256)
    th2 = 2 * np.pi * np.outer(t2, t2) / 256
    C2 = np.cos(th2)
    S2 = np.sin(th2)
    idx = (np.arange(KS) - (KS // 2)) % 256
    GcT = C2[idx, :]  # [25,256]
    GsT = S2[idx, :]

    def to_plane(a):  # [256,256] -> [128,512]
        return a.reshape(2, 128, 256).transpose(1, 0, 2).reshape(128, 512)

    Cnat = to_plane(C2)
    Snat = to_plane(S2)

    consts = {}
    consts["CS1"] = np.concatenate([C1, S1], 1)              # [128,256]
    consts["CS2"] = np.concatenate([-S1, C1], 1)             # [128,256]
    consts["GT"] = np.concatenate([GcT, GsT], 1)             # [25,512]
    zpad = np.zeros((7, 128))
    for hh in range(2):
        hsl = slice(hh * 128, (hh + 1) * 128)
        consts[f"GGr{hh}"] = np.concatenate([GcT[:, hsl], zpad, -GsT[:, hsl]], 0)  # [57,128]
        consts[f"GGs{hh}"] = np.concatenate([GsT[:, hsl], zpad, GcT[:, hsl]], 0)
    # Ghat for Y
    d = 1 + np.exp(-2j * np.pi * t2 / 256)
    G = np.exp(-2j * np.pi * np.outer(t2, idx) / 256)        # [256,25]
    Ghat = (d[:128, None] * G[:128, :]) + (d[128:, None] * G[128:, :])  # [128,25]
    Ghc, Ghs = np.real(Ghat), -np.imag(Ghat)                 # Ghat = Ghc - i*Ghs
    consts["GhT2"] = np.concatenate([Ghc.T, Ghs.T], 1)       # [25,256]
    consts["Kr"] = np.concatenate([Ghc.T, zpad, -Ghs.T], 0)  # [57,128] -> bf16
    consts["Ks"] = np.concatenate([Ghs.T, zpad, Ghc.T], 0)   # [57,128] -> bf16
    # VT moving blocks
    vm1, vm2 = [], []
    for kc in range(2):
        c = Cnat[:, kc * 256:(kc + 1) * 256]
        s = Snat[:, kc * 256:(kc + 1) * 256]
        vm1.append(np.concatenate([c, -s], 1))
        vm2.append(np.concatenate([s, c], 1))
    consts["VM1"] = np.concatenate(vm1, 1)                   # [128,1024] -> bf16
    consts["VM2"] = np.concatenate(vm2, 1)
    consts["CnS"] = Cnat / 65536.0                           # [128,512] -> bf16
    consts["SnS"] = Snat / 65536.0
    # xu row-replication selectors [128,256] = [R0 | R1]
    R = np.zeros((2, 128, 128))
    for mb in range(2):
        for p in range(128):
            R[mb, mb * 64 + p // 2, p] = 1.0
    consts["R2"] = np.concatenate([R[0], R[1]], 1)           # [128,256]
    consts["ZP256"] = np.zeros((7, 256))
    consts["ZP128"] = np.zeros((7, 128))
    return consts


CONST_SPECS = {
    "CS1": ([128, 256], F32R), "CS2": ([128, 256], F32R),
    "GT": ([25, 512], F32R),
    "GGr0": ([57, 128], F32R), "GGs0": ([57, 128], F32R),
    "GGr1": ([57, 128], F32R), "GGs1": ([57, 128], F32R),
    "GhT2": ([25, 256], F32R),
    "Kr": ([57, 128], BF16), "Ks": ([57, 128], BF16),
    "VM1": ([128, 1024], F32R), "VM2": ([128, 1024], F32R),
    "CnS": ([128, 512], F32R), "SnS": ([128, 512], F32R),
    "R2": ([128, 256], F32R),
    "ZP256": ([7, 256], F32R),
    "ZP128": ([7, 128], BF16),
}


def build_nc(n_planes=NPL, debug=False):
    nc = bacc.Bacc("TRN2", target_bir_lowering=False, debug=False, num_devices=N_CORES)

    xs_t = nc.dram_tensor("xs", [n_planes, 128, 128], F32R, kind="ExternalInput")
    kt_t = nc.dram_tensor("kt", [n_planes, KS, KS], F32R, kind="ExternalInput")
    be4_t = nc.dram_tensor("be4", [128, n_planes], F32, kind="ExternalInput")
    const_t = {n: nc.dram_tensor(n, s, dt, kind="ExternalInput") for n, (s, dt) in CONST_SPECS.items()}
    out_t = nc.dram_tensor("out", [n_planes, 256, 256], F32, kind="ExternalOutput")
    dbg_t = {}
    if debug:
        for nm, shp, dt in [("g16", [128, 256], BF16), ("a_st", [57, 256], F32),
                            ("a2st", [57, 128], BF16), ("FB16", [128, 1024], BF16),
                            ("Wp", [128, 128], F32), ("wr16", [128, 128], BF16),
                            ("ws16", [128, 128], BF16), ("u16", [128, 384], BF16),
                            ("FX16", [128, 1024], F32), ("VT16", [128, 1024], F32),
                            ("xc", [128, 256], F32), ("z_sb", [128, 256], F32)]:
            dbg_t[nm] = nc.dram_tensor(f"dbg_{nm}", shp, dt, kind="ExternalOutput")

    r = lambda ap: ap.bitcast(F32R)

    with tile.TileContext(nc) as tc:
        with (
            tc.tile_pool(name="cpool", bufs=1) as cpool,
            tc.tile_pool(name="small", bufs=3) as small,
            tc.tile_pool(name="big", bufs=2) as big,
            tc.tile_pool(name="psA", bufs=1, space="PSUM") as psA,
            tc.tile_pool(name="psB", bufs=2, space="PSUM") as psB,
        ):
            cs = {}
            for n, (s, dt) in CONST_SPECS.items():
                cs[n] = cpool.tile(s, dt, tag=n, name=f"c_{n}")
                nc.sync.dma_start(cs[n][:], const_t[n][:])
            be4sb = cpool.tile([128, n_planes], F32, tag="be4sb")
            nc.sync.dma_start(be4sb[:], be4_t[:])
            a_sts, a2sts = [], []
            for j in range(3):
                t = cpool.tile([57, 256], F32R, tag=f"a_st{j}", name=f"a_st{j}")
                nc.sync.dma_start(t[25:32, :], cs["ZP256"][:])
                a_sts.append(t)
                t2 = cpool.tile([57, 128], BF16, tag=f"a2st{j}", name=f"a2st{j}")
                nc.sync.dma_start(t2[25:32, :], cs["ZP128"][:])
                a2sts.append(t2)

            for i in range(n_planes):
                # ---- loads ----
                x_sb = small.tile([128, 128], F32R, tag="x_sb")
                nc.sync.dma_start(x_sb[:], xs_t[i])
                kt_sb = small.tile([KS, KS], F32R, tag="kt_sb")
                nc.sync.dma_start(kt_sb[:], kt_t[i])

                # ---- fft128: pzg bank = [Z | PG] is too big; use [128,512] = [pz|pg] ----
                pzg = psB.tile([128, 512], F32, tag="pzg_po")  # shared rotation with po
                pz = pzg[:, 0:256]
                pg = pzg[:, 256:512]
                nc.tensor.matmul(pz, r(x_sb[:]), r(cs["CS1"][:]), start=True, stop=True)
                z_sb = small.tile([128, 256], F32R, tag="z_sb")
                nc.scalar.copy(z_sb[:], pz)
                nc.tensor.matmul(pg, r(z_sb[:, 0:128]), r(cs["CS1"][:]), start=True, stop=False)
                nc.tensor.matmul(pg, r(z_sb[:, 128:256]), r(cs["CS2"][:]), start=False, stop=True)
                g16 = small.tile([128, 256], BF16, tag="g16")  # [gr|gs]
                nc.vector.tensor_copy(g16[:], pg)

                # ---- FB = G k G^T ----
                pa = psA.tile([25, 512], F32, tag="pa")
                nc.tensor.matmul(pa[:], r(kt_sb[:]), r(cs["GT"][:]), start=True, stop=True)
                a_st = a_sts[i % 3]  # rows [Ar; 0pad; As]
                nc.scalar.copy(a_st[0:25, :], pa[:, 0:256])
                nc.scalar.copy(a_st[32:57, :], pa[:, 256:512])

                # ---- Y = Ghat k Ghat^T ----
                pY2 = psA.tile([128, 512], F32, tag="pY2")  # [0:25,0:256]=pa2, [:,256:512]=pY
                pa2 = pY2[0:25, 0:256]
                pY = pY2[:, 256:512]
                nc.tensor.matmul(pa2, r(kt_sb[:]), r(cs["GhT2"][:]), start=True, stop=True)
                a2st = a2sts[i % 3]  # rows [A2r; 0pad; A2s]
                nc.vector.tensor_copy(a2st[0:25, :], pY2[0:25, 0:128])
                nc.vector.tensor_copy(a2st[32:57, :], pY2[0:25, 128:256])
                nc.tensor.matmul(pY[:, 0:128], cs["Kr"][:], a2st[:], start=True, stop=True)
                nc.tensor.matmul(pY[:, 128:256], cs["Ks"][:], a2st[:], start=True, stop=True)

                # FB16 [128,1024] = [FBr_h0 | FBr_h1 | FBs_h0 | FBs_h1] (component-major)
                FB16 = big.tile([128, 1024], BF16, tag="FB16")
                pfbR = psB.tile([128, 512], F32, tag="pfbR", bufs=1)
                pfbS = psB.tile([128, 512], F32, tag="pfbS", bufs=1)
                for hh in range(2):
                    hsl = slice(hh * 256, (hh + 1) * 256)
                    nc.tensor.matmul(pfbR[:, hsl], r(cs[f"GGr{hh}"][:]), r(a_st[:]), start=True, stop=True)
                    nc.tensor.matmul(pfbS[:, hsl], r(cs[f"GGs{hh}"][:]), r(a_st[:]), start=True, stop=True)
                nc.scalar.copy(FB16[:, 0:512], pfbR[:])
                nc.scalar.copy(FB16[:, 512:1024], pfbS[:])

                # ---- W = sum4 |FB|^2 ----
                sq = big.tile([128, 1024], BF16, tag="sq")
                nc.vector.tensor_mul(sq[:], FB16[:], FB16[:])
                tt = big.tile([128, 512], BF16, tag="tt")  # F2B = FBr^2 + FBs^2, [F2B_h0|F2B_h1]
                nc.vector.tensor_add(tt[:], sq[:, 0:512], sq[:, 512:1024])
                w2 = small.tile([128, 256], BF16, tag="w2")
                ttv = tt[:].rearrange("p (h v f) -> p h v f", h=2, v=2)
                nc.vector.tensor_add(w2[:].rearrange("p (h f) -> p h f", h=2), ttv[:, :, 0, :], ttv[:, :, 1, :])
                Wp = small.tile([128, 128], F32, tag="Wp")
                nc.vector.tensor_add(Wp[:], w2[:, 0:128], w2[:, 128:256])

                # ---- wt = (4 - Y)/(W + 4be) ----
                den = small.tile([128, 128], F32, tag="den")
                nc.scalar.add(den[:], Wp[:], be4sb[:, i:i + 1])
                dinv = small.tile([128, 128], F32, tag="dinv")
                nc.vector.reciprocal_approx_fast(dinv[:], den[:])
                wt4 = small.tile([128, 128], F32, tag="wt4")
                nc.scalar.activation(wt4[:], pY[:, 0:128], mybir.ActivationFunctionType.Copy, bias=4.0, scale=-1.0)
                wr16 = small.tile([128, 128], BF16, tag="wr16")
                nc.vector.tensor_mul(wr16[:], wt4[:], dinv[:])
                ws16 = small.tile([128, 128], BF16, tag="ws16")
                nc.vector.scalar_tensor_tensor(ws16[:], pY[:, 128:256], -1.0, dinv[:], MULT, MULT)

                # ---- u = g*wt:  u16 [128,384] = [ur | us | ur] ----
                u16 = small.tile([128, 384], BF16, tag="u16")
                gr = g16[:, 0:128]
                gs = g16[:, 128:256]
                mu = small.tile([128, 128], BF16, tag="mu")
                nc.vector.tensor_mul(mu[:], gr, wr16[:])
                t1 = small.tile([128, 128], BF16, tag="t1u")
                nc.vector.scalar_tensor_tensor(t1[:], gs, -1.0, ws16[:], MULT, MULT)
                nc.vector.tensor_add(u16[:, 0:128], mu[:], t1[:])   # ur = gr*wr - gs*ws
                nc.gpsimd.tensor_copy(u16[:, 256:384], u16[:, 0:128])
                mu2 = small.tile([128, 128], BF16, tag="mu2")
                nc.vector.tensor_mul(mu2[:], gr, ws16[:])
                t2 = small.tile([128, 128], BF16, tag="t2u")
                nc.vector.tensor_mul(t2[:], gs, wr16[:])
                nc.vector.tensor_add(u16[:, 128:256], mu2[:], t2[:])  # us = gr*ws + gs*wr

                # ---- FX = conj(FB)*tile(u) ----
                # FX16 [128,1024] = [FXr_h0 | FXr_h1 | FXs_h0 | FXs_h1] (component-major)
                FX16 = big.tile([128, 1024], F32R, tag="FX16")
                P = big.tile([128, 1024], BF16, tag="Pfx")
                Q = big.tile([128, 1024], BF16, tag="Qfx")
                urep = u16[:, 0:256].rearrange("p (c f) -> p c f", c=2)
                urep = urep.unsqueeze(2).broadcast_to([128, 2, 4, 128])
                srep = u16[:, 128:384].rearrange("p (c f) -> p c f", c=2)
                srep = srep.unsqueeze(2).broadcast_to([128, 2, 4, 128])
                fb_v = FB16[:].rearrange("p (c q f) -> p c q f", c=2, q=4)
                nc.vector.tensor_tensor(P[:].rearrange("p (c q f) -> p c q f", c=2, q=4), fb_v, urep, MULT)
                nc.vector.tensor_add(FX16[:, 0:512], P[:, 0:512], P[:, 512:1024])
                nc.vector.tensor_tensor(Q[:].rearrange("p (c q f) -> p c q f", c=2, q=4), fb_v, srep, MULT)
                nc.vector.scalar_tensor_tensor(FX16[:, 512:1024], Q[:, 512:1024], -1.0, Q[:, 0:512], MULT, ADD)

                # ---- ifft stage 1: VT16 [128,1024] = [VTr_f0 | VTs_f0 | VTr_f1 | VTs_f1] ----
                VT16 = big.tile([128, 1024], F32R, tag="VT16")
                for fb in range(2):
                    pvt = psB.tile([128, 512], F32, tag="pvt")
                    for kc in range(2):
                        fxr = FX16[:, kc * 256 + fb * 128: kc * 256 + (fb + 1) * 128]
                        fxs = FX16[:, 512 + kc * 256 + fb * 128: 512 + kc * 256 + (fb + 1) * 128]
                        nc.tensor.matmul(pvt[:], fxr, cs["VM1"][:, kc * 512:(kc + 1) * 512], start=(kc == 0), stop=False)
                        nc.tensor.matmul(pvt[:], fxs, cs["VM2"][:, kc * 512:(kc + 1) * 512], start=False, stop=(kc == 1))
                    nc.scalar.copy(VT16[:, fb * 512:(fb + 1) * 512], pvt[:])

                # ---- ifft stage 2 + xu + store ----
                xc = small.tile([128, 256], F32R, tag="xc")
                nc.vector.tensor_copy(
                    xc[:].rearrange("p (f two) -> p f two", two=2),
                    x_sb[:].unsqueeze(2).broadcast_to([128, 128, 2]),
                )
                po = psB.tile([128, 512], F32, tag="pzg_po")
                for mb in range(2):
                    osl = slice(mb * 256, (mb + 1) * 256)
                    for fb in range(2):
                        vtr = VT16[:, fb * 512 + mb * 128: fb * 512 + (mb + 1) * 128]
                        vts = VT16[:, fb * 512 + 256 + mb * 128: fb * 512 + 256 + (mb + 1) * 128]
                        csl = slice(fb * 256, (fb + 1) * 256)
                        nc.tensor.matmul(po[:, osl], vtr, cs["CnS"][:, csl], start=(fb == 0), stop=False)
                        nc.tensor.matmul(po[:, osl], vts, cs["SnS"][:, csl], start=False, stop=False)
                    nc.tensor.matmul(po[:, osl], r(cs["R2"][:, mb * 128:(mb + 1) * 128]), r(xc[:]), start=False, stop=True)
                out_sb = big.tile([128, 512], F32, tag="out_sb")
                nc.scalar.copy(out_sb[:], po[:])
                nc.sync.dma_start(
                    out_t[i].rearrange("(hb p) f -> p hb f", p=128),
                    out_sb[:].rearrange("p (hb f) -> p hb f", hb=2),
                )
                if debug and i == 0:
                    for nm, t in [("g16", g16), ("a_st", a_st), ("a2st", a2st),
                                  ("FB16", FB16), ("Wp", Wp), ("wr16", wr16),
                                  ("ws16", ws16), ("u16", u16), ("FX16", FX16),
                                  ("VT16", VT16), ("xc", xc), ("z_sb", z_sb)]:
                        nc.sync.dma_start(dbg_t[nm][:], t[:].bitcast(dbg_t[nm].dtype) if t[:].dtype != dbg_t[nm].dtype else t[:])

    nc.compile()
    return nc


@functools.lru_cache(maxsize=2)
def _built(n_planes=NPL):
    return build_nc(n_planes)


def _round_fp32r(a):
    """fp32 -> fp32r (11-bit mantissa, RNE), as float32."""
    u = np.ascontiguousarray(a, dtype=np.float32).view(np.uint32)
    r = (u + 0x7FF + ((u >> 12) & 1)) & np.uint32(0xFFFFF000)
    return r.astype(np.uint32).view(np.float32)


def make_in_maps(x, k, alpha, n_planes=NPL, n_cores=N_CORES):
    consts_f64 = _host_consts()
    consts = {}
    for n, (s, dt) in CONST_SPECS.items():
        a = consts_f64[n]
        if dt == BF16:
            consts[n] = np.ascontiguousarray(a.astype(ml_dtypes.bfloat16))
        elif dt == F32R:
            consts[n] = _round_fp32r(a)
        else:
            consts[n] = np.ascontiguousarray(a, dtype=np.float32)
    alpha_c = alpha.reshape(-1).astype(np.float64)  # [64]
    be = (1.0 / (1.0 + np.exp(-(alpha_c - 9.0))) + 1e-3).astype(np.float32)
    cpc = n_planes // 4  # channels per core
    in_maps = []
    for ci in range(n_cores):
        chs = slice(ci * cpc, (ci + 1) * cpc)
        xs = _round_fp32r(x[:, chs].transpose(1, 0, 2, 3).reshape(n_planes, 128, 128))
        kt = _round_fp32r(k[:, chs].transpose(1, 0, 3, 2).reshape(n_planes, KS, KS))
        be_pl = np.repeat(be[chs], 4)  # plane order: (c_loc, b)
        be4 = np.broadcast_to(4.0 * be_pl, (128, n_planes)).astype(np.float32).copy()
        m = {"xs": xs, "kt": kt, "be4": be4}
        m.update(consts)
        in_maps.append(m)
    return in_maps


def kernel(x, k, alpha, sf=2, **_ignored):
    x = np.asarray(x, dtype=np.float32)
    k = np.asarray(k, dtype=np.float32)
    alpha = np.asarray(alpha, dtype=np.float32)
    assert int(sf) == 2 and x.shape == (4, 64, 128, 128) and k.shape == (4, 64, KS, KS)

    nc = _built(NPL)
    in_maps = make_in_maps(x, k, alpha)
    res = run_bass_kernel_spmd(nc, in_maps, core_ids=list(range(N_CORES)))
    out = np.empty((4, 64, 256, 256), np.float32)
    cpc = NPL // 4
    for ci in range(N_CORES):
        o = res.results[ci]["out"].reshape(cpc, 4, 256, 256).transpose(1, 0, 2, 3)
        out[:, ci * cpc:(ci + 1) * cpc] = o
    return out


if __name__ == "__main__":
    rng = np.random.default_rng(0)
    x = rng.standard_normal((4, 64, 128, 128), dtype=np.float32)
    k = rng.random((4, 64, KS, KS), dtype=np.float32)
    alpha = np.zeros((1, 64, 1, 1), np.float32)
    out = kernel(x, k, alpha, 2)
    print("out", out.shape, out.dtype, float(np.abs(out).max()))


# revision 14
# speedup vs baseline: 2.2245x; 1.3017x over previous
"""Trainium2 Bass kernel for ConvReverseDataNet (USRNet-style FFT data step).

Math (per (b,c) plane, sf=2, validated in numpy mock):
  g   = fft2_128(x)                                 (128x128 complex)
  FB  = G @ k @ G^T, G = F256[:, roll_idx]          (256x256 complex)
  W   = sum_{4 blocks} |FB|^2                       (128x128 real)
  Y   = sum_{4 blocks} FB*DD = Ghat @ k @ Ghat^T    (128x128 complex, on PE)
        where Ghat[u,kap] = sum_a d[u+128a] G[u+128a, kap], d[t]=1+exp(-2pi i t/256)
  wt  = (4 - Y) / (W + 4*be)
  u   = g * wt
  FX  = conj(FB) * tile(u)
  out = real(ifft2_256(FX)) + nn_upsample(x)        [ifft2(tile(g)*DD) == xu identity]
All complex X stored as (Xr, Xs) with X = Xr - i*Xs.
256-row planes in SBUF as [128, 512]: [p, hb*256+f] = plane[hb*128+p, f].

Perf notes: matmuls in float32r (1 cyc/row at free>=256) or bf16 (1 cyc/row);
elementwise mostly bf16 on DVE, balanced against Act (copies) and Pool/gpsimd.

Sharding: 256 (b,c) planes over 8 cores; core ci gets channels ci*8..ci*8+7 x 4 batches.
"""

import functools
import sys

import ml_dtypes
import numpy as np

if "/opt/trn_rl_repo" not in sys.path:
    sys.path.insert(0, "/opt/trn_rl_repo")

from concourse import bacc, bass, mybir, tile  # noqa: E402
from concourse.bass_utils import run_bass_kernel_spmd  # noqa: E402

F32 = mybir.dt.float32
F32R = mybir.dt.float32r
BF16 = mybir.dt.bfloat16
MULT = mybir.AluOpType.mult
ADD = mybir.AluOpType.add

N_CORES = 8
NPL = 32  # planes per core
KS = 25


def _host_consts():
    t1 = np.arange(128)
    th1 = 2 * np.pi * np.outer(t1, t1) / 128
    C1 = np.cos(th1)
    S1 = np.sin(th1)
    t2 = np.arange(256)
    th2 = 2 * np.pi * np.outer(t2, t2) / 256
    C2 = np.cos(th2)
    S2 = np.sin(th2)
    idx = (np.arange(KS) - (KS // 2)) % 256
    GcT = C2[idx, :]  # [25,256]
    GsT = S2[idx, :]

    def to_plane(a):  # [256,256] -> [128,512]
        return a.reshape(2, 128, 256).transpose(1, 0, 2).reshape(128, 512)

    Cnat = to_plane(C2)
    Snat = to_plane(S2)

    consts = {}
    consts["CS1"] = np.concatenate([C1, S1], 1)              # [128,256]
    consts["CS2"] = np.concatenate([-S1, C1], 1)             # [128,256]
    consts["GT"] = np.concatenate([GcT, GsT], 1)             # [25,512]
    zpad = np.zeros((7, 128))
    for hh in range(2):
        hsl = slice(hh * 128, (hh + 1) * 128)
        consts[f"GGr{hh}"] = np.concatenate([GcT[:, hsl], zpad, -GsT[:, hsl]], 0)  # [57,128]
        consts[f"GGs{hh}"] = np.concatenate([GsT[:, hsl], zpad, GcT[:, hsl]], 0)
    # Ghat for Y
    d = 1 + np.exp(-2j * np.pi * t2 / 256)
    G = np.exp(-2j * np.pi * np.outer(t2, idx) / 256)        # [256,25]
    Ghat = (d[:128, None] * G[:128, :]) + (d[128:, None] * G[128:, :])  # [128,25]
    Ghc, Ghs = np.real(Ghat), -np.imag(Ghat)                 # Ghat = Ghc - i*Ghs
    consts["GhT2"] = np.concatenate([Ghc.T, Ghs.T], 1)       # [25,256]
    consts["Kr"] = np.concatenate([Ghc.T, zpad, -Ghs.T], 0)  # [57,128] -> bf16
    consts["Ks"] = np.concatenate([Ghs.T, zpad, Ghc.T], 0)   # [57,128] -> bf16
    # VT moving blocks
    vm1, vm2 = [], []
    for kc in range(2):
        c = Cnat[:, kc * 256:(kc + 1) * 256]
        s = Snat[:, kc * 256:(kc + 1) * 256]
        vm1.append(np.concatenate([c, -s], 1))
        vm2.append(np.concatenate([s, c], 1))
    consts["VM1"] = np.concatenate(vm1, 1)                   # [128,1024] -> bf16
    consts["VM2"] = np.concatenate(vm2, 1)
    consts["CnS"] = Cnat / 65536.0                           # [128,512] -> bf16
    consts["SnS"] = Snat / 65536.0
    # xu row-replication selectors [128,256] = [R0 | R1]
    R = np.zeros((2, 128, 128))
    for mb in range(2):
        for p in range(128):
            R[mb, mb * 64 + p // 2, p] = 1.0
    consts["R2"] = np.concatenate([R[0], R[1]], 1)           # [128,256]
    consts["ZP256"] = np.zeros((7, 256))
    consts["ZP128"] = np.zeros((7, 128))
    return consts


CONST_SPECS = {
    "CS1": ([128, 256], F32R), "CS2": ([128, 256], F32R),
    "GT": ([25, 512], F32R),
    "GGr0": ([57, 128], F32R), "GGs0": ([57, 128], F32R),
    "GGr1": ([57, 128], F32R), "GGs1": ([57, 128], F32R),
    "GhT2": ([25, 256], F32R),
    "Kr": ([57, 128], BF16), "Ks": ([57, 128], BF16),
    "VM1": ([128, 1024], F32R), "VM2": ([128, 1024], F32R),
    "CnS": ([128, 512], F32R), "SnS": ([128, 512], F32R),
    "R2": ([128, 256], F32R),
    "ZP256": ([7, 256], F32R),
    "ZP128": ([7, 128], BF16),
}


def build_nc(n_planes=NPL, debug=False):
    nc = bacc.Bacc("TRN2", target_bir_lowering=False, debug=False, num_devices=N_CORES)

    xs_t = nc.dram_tensor("xs", [n_planes, 128, 128], F32R, kind="ExternalInput")
    kt_t = nc.dram_tensor("kt", [n_planes, KS, KS], F32R, kind="ExternalInput")
    be4_t = nc.dram_tensor("be4", [128, n_planes], F32, kind="ExternalInput")
    const_t = {n: nc.dram_tensor(n, s, dt, kind="ExternalInput") for n, (s, dt) in CONST_SPECS.items()}
    out_t = nc.dram_tensor("out", [n_planes, 256, 256], F32, kind="ExternalOutput")
    dbg_t = {}
    if debug:
        for nm, shp, dt in [("g16", [128, 256], BF16), ("a_st", [57, 256], F32),
                            ("a2st", [57, 128], BF16), ("FB16", [128, 1024], BF16),
                            ("Wp", [128, 128], F32), ("wr16", [128, 128], BF16),
                            ("ws16", [128, 128], BF16), ("u16", [128, 384], BF16),
                            ("FX16", [128, 1024], F32), ("VT16", [128, 1024], F32),
                            ("xc", [128, 256], F32), ("z_sb", [128, 256], F32)]:
            dbg_t[nm] = nc.dram_tensor(f"dbg_{nm}", shp, dt, kind="ExternalOutput")

    r = lambda ap: ap.bitcast(F32R)

    with tile.TileContext(nc) as tc:
        with (
            tc.tile_pool(name="cpool", bufs=1) as cpool,
            tc.tile_pool(name="small", bufs=3) as small,
            tc.tile_pool(name="big", bufs=2) as big,
            tc.tile_pool(name="psA", bufs=1, space="PSUM") as psA,
            tc.tile_pool(name="psB", bufs=2, space="PSUM") as psB,
        ):
            cs = {}
            for n, (s, dt) in CONST_SPECS.items():
                cs[n] = cpool.tile(s, dt, tag=n, name=f"c_{n}")
                nc.sync.dma_start(cs[n][:], const_t[n][:])
            be4sb = cpool.tile([128, n_planes], F32, tag="be4sb")
            nc.sync.dma_start(be4sb[:], be4_t[:])
            # PE warm-up: ~3.5us of back-to-back matmuls to open the HAM clock gate
            wp = psB.tile([128, 512], F32, tag="pzg", bufs=1, name="warm")
            for w in range(24):
                nc.tensor.matmul(wp[:, 0:256], r(cs["CS1"][:, 0:128]), r(cs["CS1"][:]), start=True, stop=True)
            a_sts, a2sts = [], []
            for j in range(3):
                t = cpool.tile([57, 256], F32R, tag=f"a_st{j}", name=f"a_st{j}")
                nc.sync.dma_start(t[25:32, :], cs["ZP256"][:])
                a_sts.append(t)
                t2 = cpool.tile([57, 128], BF16, tag=f"a2st{j}", name=f"a2st{j}")
                nc.sync.dma_start(t2[25:32, :], cs["ZP128"][:])
                a2sts.append(t2)

            for i in range(n_planes):
                # ---- loads ----
                x_sb = small.tile([128, 128], F32R, tag="x_sb")
                nc.sync.dma_start(x_sb[:], xs_t[i])
                kt_sb = small.tile([KS, KS], F32R, tag="kt_sb")
                nc.sync.dma_start(kt_sb[:], kt_t[i])
                xc = small.tile([128, 256], F32R, tag="xc")
                nc.vector.tensor_copy(
                    xc[:].rearrange("p (f two) -> p f two", two=2),
                    x_sb[:].unsqueeze(2).broadcast_to([128, 128, 2]),
                )

                # ---- fft128: pzg bank = [Z | PG] is too big; use [128,512] = [pz|pg] ----
                pzg = psB.tile([128, 512], F32, tag="pzg", bufs=1)
                pz = pzg[:, 0:256]
                pg = pzg[:, 256:512]
                nc.tensor.matmul(pz, r(x_sb[:]), r(cs["CS1"][:]), start=True, stop=True)
                z_sb = small.tile([128, 256], F32R, tag="z_sb")
                nc.scalar.copy(z_sb[:], pz)
                nc.tensor.matmul(pg, r(z_sb[:, 0:128]), r(cs["CS1"][:]), start=True, stop=False)
                nc.tensor.matmul(pg, r(z_sb[:, 128:256]), r(cs["CS2"][:]), start=False, stop=True)
                g16 = small.tile([128, 256], BF16, tag="g16")  # [gr|gs]
                nc.vector.tensor_copy(g16[:], pg)

                # ---- FB = G k G^T ----
                pa = psA.tile([25, 512], F32, tag="pa")
                nc.tensor.matmul(pa[:], r(kt_sb[:]), r(cs["GT"][:]), start=True, stop=True)
                a_st = a_sts[i % 3]  # rows [Ar; 0pad; As]
                nc.scalar.copy(a_st[0:25, :], pa[:, 0:256])
                nc.scalar.copy(a_st[32:57, :], pa[:, 256:512])

                # ---- Y = Ghat k Ghat^T ----
                pY2 = psA.tile([128, 512], F32, tag="pY2")  # [0:25,0:256]=pa2, [:,256:512]=pY
                pa2 = pY2[0:25, 0:256]
                pY = pY2[:, 256:512]
                nc.tensor.matmul(pa2, r(kt_sb[:]), r(cs["GhT2"][:]), start=True, stop=True)
                a2st = a2sts[i % 3]  # rows [A2r; 0pad; A2s]
                nc.vector.tensor_copy(a2st[0:25, :], pY2[0:25, 0:128])
                nc.vector.tensor_copy(a2st[32:57, :], pY2[0:25, 128:256])
                nc.tensor.matmul(pY[:, 0:128], cs["Kr"][:], a2st[:], start=True, stop=True)
                nc.tensor.matmul(pY[:, 128:256], cs["Ks"][:], a2st[:], start=True, stop=True)
                wt4 = small.tile([128, 128], F32, tag="wt4")
                nc.scalar.activation(wt4[:], pY[:, 0:128], mybir.ActivationFunctionType.Copy, bias=4.0, scale=-1.0)
                Ys16 = small.tile([128, 128], BF16, tag="Ys16")
                nc.scalar.copy(Ys16[:], pY[:, 128:256])

                # FB16 [128,1024] = [FBr_h0 | FBr_h1 | FBs_h0 | FBs_h1] (component-major)
                FB16 = big.tile([128, 1024], BF16, tag="FB16")
                pfbR = psB.tile([128, 512], F32, tag="pfbR", bufs=1)
                pfbS = psB.tile([128, 512], F32, tag="pfbS", bufs=1)
                for hh in range(2):
                    hsl = slice(hh * 256, (hh + 1) * 256)
                    nc.tensor.matmul(pfbR[:, hsl], r(cs[f"GGr{hh}"][:]), r(a_st[:]), start=True, stop=True)
                    nc.tensor.matmul(pfbS[:, hsl], r(cs[f"GGs{hh}"][:]), r(a_st[:]), start=True, stop=True)
                nc.scalar.copy(FB16[:, 0:512], pfbR[:])
                nc.scalar.copy(FB16[:, 512:1024], pfbS[:])

                # ---- W = sum4 |FB|^2 ----
                sq = big.tile([128, 1024], BF16, tag="sq")
                nc.vector.tensor_mul(sq[:], FB16[:], FB16[:])
                tt = big.tile([128, 512], BF16, tag="tt")  # F2B = FBr^2 + FBs^2, [F2B_h0|F2B_h1]
                nc.vector.tensor_add(tt[:], sq[:, 0:512], sq[:, 512:1024])
                w2 = small.tile([128, 256], BF16, tag="w2")
                ttv = tt[:].rearrange("p (h v f) -> p h v f", h=2, v=2)
                nc.vector.tensor_add(w2[:].rearrange("p (h f) -> p h f", h=2), ttv[:, :, 0, :], ttv[:, :, 1, :])
                Wp = small.tile([128, 128], F32, tag="Wp")
                nc.vector.tensor_add(Wp[:], w2[:, 0:128], w2[:, 128:256])

                # ---- wt = (4 - Y)/(W + 4be) ----
                den = small.tile([128, 128], F32, tag="den")
                nc.scalar.add(den[:], Wp[:], be4sb[:, i:i + 1])
                dinv = small.tile([128, 128], F32, tag="dinv")
                nc.vector.reciprocal_approx_fast(dinv[:], den[:])
                wr16 = small.tile([128, 128], BF16, tag="wr16")
                nc.vector.tensor_mul(wr16[:], wt4[:], dinv[:])
                ws16 = small.tile([128, 128], BF16, tag="ws16")
                nc.vector.scalar_tensor_tensor(ws16[:], Ys16[:], -1.0, dinv[:], MULT, MULT)

                # ---- u = g*wt:  u16 [128,384] = [ur | us | ur] ----
                u16 = small.tile([128, 384], BF16, tag="u16")
                gr = g16[:, 0:128]
                gs = g16[:, 128:256]
                mu = small.tile([128, 128], BF16, tag="mu")
                nc.vector.tensor_mul(mu[:], gr, wr16[:])
                t1 = small.tile([128, 128], BF16, tag="t1u")
                nc.vector.scalar_tensor_tensor(t1[:], gs, -1.0, ws16[:], MULT, MULT)
                nc.vector.tensor_add(u16[:, 0:128], mu[:], t1[:])   # ur = gr*wr - gs*ws
                nc.gpsimd.tensor_copy(u16[:, 256:384], u16[:, 0:128])
                mu2 = small.tile([128, 128], BF16, tag="mu2")
                nc.vector.tensor_mul(mu2[:], gr, ws16[:])
                t2 = small.tile([128, 128], BF16, tag="t2u")
                nc.vector.tensor_mul(t2[:], gs, wr16[:])
                nc.vector.tensor_add(u16[:, 128:256], mu2[:], t2[:])  # us = gr*ws + gs*wr

                # ---- FX = conj(FB)*tile(u) ----
                # FX16 [128,1024] = [FXr_h0 | FXr_h1 | FXs_h0 | FXs_h1] (component-major)
                FX16 = big.tile([128, 1024], F32R, tag="FX16")
                P = big.tile([128, 1024], BF16, tag="Pfx")
                Q = big.tile([128, 1024], BF16, tag="Qfx")
                urep = u16[:, 0:256].rearrange("p (c f) -> p c f", c=2)
                urep = urep.unsqueeze(2).broadcast_to([128, 2, 4, 128])
                srep = u16[:, 128:384].rearrange("p (c f) -> p c f", c=2)
                srep = srep.unsqueeze(2).broadcast_to([128, 2, 4, 128])
                fb_v = FB16[:].rearrange("p (c q f) -> p c q f", c=2, q=4)
                nc.vector.tensor_tensor(P[:].rearrange("p (c q f) -> p c q f", c=2, q=4), fb_v, urep, MULT)
                nc.vector.tensor_add(FX16[:, 0:512], P[:, 0:512], P[:, 512:1024])
                nc.vector.tensor_tensor(Q[:].rearrange("p (c q f) -> p c q f", c=2, q=4), fb_v, srep, MULT)
                nc.vector.scalar_tensor_tensor(FX16[:, 512:1024], Q[:, 512:1024], -1.0, Q[:, 0:512], MULT, ADD)

                # ---- ifft stage 1: VT16 [128,1024] = [VTr_f0 | VTs_f0 | VTr_f1 | VTs_f1] ----
                VT16 = big.tile([128, 1024], F32R, tag="VT16")
                for fb in range(2):
                    pvt = psB.tile([128, 512], F32, tag="pvt", bufs=1)
                    for kc in range(2):
                        fxr = FX16[:, kc * 256 + fb * 128: kc * 256 + (fb + 1) * 128]
                        fxs = FX16[:, 512 + kc * 256 + fb * 128: 512 + kc * 256 + (fb + 1) * 128]
                        nc.tensor.matmul(pvt[:], fxr, cs["VM1"][:, kc * 512:(kc + 1) * 512], start=(kc == 0), stop=False)
                        nc.tensor.matmul(pvt[:], fxs, cs["VM2"][:, kc * 512:(kc + 1) * 512], start=False, stop=(kc == 1))
                    nc.scalar.copy(VT16[:, fb * 512:(fb + 1) * 512], pvt[:])

                # ---- ifft stage 2 + xu + store ----
                po = psB.tile([128, 512], F32, tag="po", bufs=2)
                for mb in range(2):
                    osl = slice(mb * 256, (mb + 1) * 256)
                    for fb in range(2):
                        vtr = VT16[:, fb * 512 + mb * 128: fb * 512 + (mb + 1) * 128]
                        vts = VT16[:, fb * 512 + 256 + mb * 128: fb * 512 + 256 + (mb + 1) * 128]
                        csl = slice(fb * 256, (fb + 1) * 256)
                        nc.tensor.matmul(po[:, osl], vtr, cs["CnS"][:, csl], start=(fb == 0), stop=False)
                        nc.tensor.matmul(po[:, osl], vts, cs["SnS"][:, csl], start=False, stop=False)
                    nc.tensor.matmul(po[:, osl], r(cs["R2"][:, mb * 128:(mb + 1) * 128]), r(xc[:]), start=False, stop=True)
                out_sb = big.tile([128, 512], F32, tag="out_sb")
                nc.scalar.copy(out_sb[:], po[:])
                nc.sync.dma_start(
                    out_t[i].rearrange("(hb p) f -> p hb f", p=128),
                    out_sb[:].rearrange("p (hb f) -> p hb f", hb=2),
                )
                if debug and i == 0:
                    for nm, t in [("g16", g16), ("a_st", a_st), ("a2st", a2st),
                                  ("FB16", FB16), ("Wp", Wp), ("wr16", wr16),
                                  ("ws16", ws16), ("u16", u16), ("FX16", FX16),
                                  ("VT16", VT16), ("xc", xc), ("z_sb", z_sb)]:
                        nc.sync.dma_start(dbg_t[nm][:], t[:].bitcast(dbg_t[nm].dtype) if t[:].dtype != dbg_t[nm].dtype else t[:])

    nc.compile()
    return nc


@functools.lru_cache(maxsize=2)
def _built(n_planes=NPL):
    return build_nc(n_planes)


def _round_fp32r(a):
    """fp32 -> fp32r (11-bit mantissa, RNE), as float32."""
    u = np.ascontiguousarray(a, dtype=np.float32).view(np.uint32)
    r = (u + 0x7FF + ((u >> 12) & 1)) & np.uint32(0xFFFFF000)
    return r.astype(np.uint32).view(np.float32)


def make_in_maps(x, k, alpha, n_planes=NPL, n_cores=N_CORES):
    consts_f64 = _host_consts()
    consts = {}
    for n, (s, dt) in CONST_SPECS.items():
        a = consts_f64[n]
        if dt == BF16:
            consts[n] = np.ascontiguousarray(a.astype(ml_dtypes.bfloat16))
        elif dt == F32R:
            consts[n] = _round_fp32r(a)
        else:
            consts[n] = np.ascontiguousarray(a, dtype=np.float32)
    alpha_c = alpha.reshape(-1).astype(np.float64)  # [64]
    be = (1.0 / (1.0 + np.exp(-(alpha_c - 9.0))) + 1e-3).astype(np.float32)
    cpc = n_planes // 4  # channels per core
    in_maps = []
    for ci in range(n_cores):
        chs = slice(ci * cpc, (ci + 1) * cpc)
        xs = _round_fp32r(x[:, chs].transpose(1, 0, 2, 3).reshape(n_planes, 128, 128))
        kt = _round_fp32r(k[:, chs].transpose(1, 0, 3, 2).reshape(n_planes, KS, KS))
        be_pl = np.repeat(be[chs], 4)  # plane order: (c_loc, b)
        be4 = np.broadcast_to(4.0 * be_pl, (128, n_planes)).astype(np.float32).copy()
        m = {"xs": xs, "kt": kt, "be4": be4}
        m.update(consts)
        in_maps.append(m)
    return in_maps


def kernel(x, k, alpha, sf=2, **_ignored):
    x = np.asarray(x, dtype=np.float32)
    k = np.asarray(k, dtype=np.float32)
    alpha = np.asarray(alpha, dtype=np.float32)
    assert int(sf) == 2 and x.shape == (4, 64, 128, 128) and k.shape == (4, 64, KS, KS)

    nc = _built(NPL)
    in_maps = make_in_maps(x, k, alpha)
    res = run_bass_kernel_spmd(nc, in_maps, core_ids=list(range(N_CORES)))
    out = np.empty((4, 64, 256, 256), np.float32)
    cpc = NPL // 4
    for ci in range(N_CORES):
        o = res.results[ci]["out"].reshape(cpc, 4, 256, 256).transpose(1, 0, 2, 3)
        out[:, ci * cpc:(ci + 1) * cpc] = o
    return out


if __name__ == "__main__":
    rng = np.random.default_rng(0)
    x = rng.standard_normal((4, 64, 128, 128), dtype=np.float32)
    k = rng.random((4, 64, KS, KS), dtype=np.float32)
    alpha = np.zeros((1, 64, 1, 1), np.float32)
    out = kernel(x, k, alpha, 2)
    print("out", out.shape, out.dtype, float(np.abs(out).max()))
